# revision 1
# baseline (speedup 1.0000x reference)
"""DGP-RF embeddings TRN2 kernel v2 — engine-balanced elementwise pipeline.

Differences vs v1 (kernel.py):
- PSUM evictions moved off the ACT engine (DVE evicts m, Pool evicts v);
  ACT does only transcendentals + Square.
- z-form moment math: r = exp(-L/2) = rsqrt(v); z = m*r; wp = L - z^2;
  sphi2 = exp(wp/2 + ln(2/sqrt(2pi))) = 2*sqrt(v)*phi(z).
- Phi never materialised: e1 = erf+1 = 2*Phi (one 4x-mode tensor_scalar);
  p2' = m*e1 = 2*m*Phi; bb' = v*(e+1) = 2*v*Phi (Pool STT, fused +1).
- ey2 never materialised: vo = 0.5*WA@aa' + 0.5*WA@bb' - 0.25*Wmu2@m2sq'
  (scale factors folded into host weights, adds 8 matmuls/tile on idle PE).
- m2sq on ACT Square (square lives in every ACT table set: no table load).
- bf16 inputs/weights for L1 (halves DMA traffic vs fp32r).

Per-core layout unchanged: features on partitions, points on free axis;
host sorts points by segment so the segment sum is a strided group-8
free-axis reduction.
"""

import numpy as np
import ml_dtypes

import concourse.bacc as bacc
import concourse.mybir as mybir
import concourse.tile as tile
from concourse import bass_utils
from concourse.tile import add_dep_helper

F32 = mybir.dt.float32
BF16 = mybir.dt.bfloat16
AF = mybir.ActivationFunctionType
ALU = mybir.AluOpType

N, S, D_IN, NUM_RF, D_OUT = 131072, 16384, 256, 1024, 128
NCORES = 8
NPC = N // NCORES
SPC = S // NCORES
K = N // S                  # 8 points per segment

PT = 256                    # points per tile
RFB = NUM_RF // 128         # 8 rf blocks
FW = RFB * PT               # elementwise width per tile (2048)
SEGT = PT // K              # segments per tile (32)
GRP = 2                     # tiles per ACT table-set window

INV_SQRT2 = float(1.0 / np.sqrt(2.0))
LN_2_OVER_SQRT2PI = float(np.log(2.0 / np.sqrt(2.0 * np.pi)))  # -0.2257913


def _patched_act_tables(arch, *a, **kw):
    import concourse.hw_specs as hw_specs
    tabs = hw_specs.get_activation_tables(arch, *a, **kw)
    exp = mybir.ActivationFunctionType.Exp
    ln = mybir.ActivationFunctionType.Ln
    if "natural_log_exp_and_others" in tabs:
        if "exp_and_others" in tabs:
            tabs["exp_and_others"] = tabs["exp_and_others"] - {exp}
        if "natural_log" in tabs:
            tabs["natural_log"] = tabs["natural_log"] - {ln}
    return tabs


def build_program(npc=NPC, passes=1):
    nt = npc // PT
    spc = npc // K
    nc = bacc.Bacc("TRN2", target_bir_lowering=False, debug=False,
                   num_devices=NCORES)

    xt_d = nc.dram_tensor("xt", [2, 128, npc], BF16, kind="ExternalInput")
    x2t_d = nc.dram_tensor("x2t", [2, 128, npc], BF16, kind="ExternalInput")
    w1mu_d = nc.dram_tensor("w1mu", [2, 128, NUM_RF], BF16, kind="ExternalInput")
    w1var_d = nc.dram_tensor("w1var", [2, 128, NUM_RF], BF16, kind="ExternalInput")
    w2mu_d = nc.dram_tensor("w2mu", [RFB, 128, D_OUT], BF16, kind="ExternalInput")
    wa_d = nc.dram_tensor("wa", [RFB, 128, D_OUT], BF16, kind="ExternalInput")
    wbn_d = nc.dram_tensor("wbn", [RFB, 128, D_OUT], BF16, kind="ExternalInput")
    ev_d = nc.dram_tensor("ev", [128, spc], F32, kind="ExternalOutput")
    em_d = nc.dram_tensor("em", [128, spc], F32, kind="ExternalOutput")

    for val in (0.0, 1e-8, LN_2_OVER_SQRT2PI):
        t = nc.alloc_sbuf_tensor(f"const-f32-{val}", [128, 1], F32)
        nc.gpsimd.memset(t.ap(), val)
        nc.const_aps.aps[(F32, val)] = t.ap()
    nc.all_engine_barrier()

    with tile.TileContext(nc) as tc:
        import contextlib
        with contextlib.ExitStack() as ctx:
            pw = ctx.enter_context(tc.tile_pool(name="w", bufs=1))
            px = ctx.enter_context(tc.tile_pool(name="x", bufs=3))
            # tensors that live across the one-window software pipeline
            pcr = ctx.enter_context(tc.tile_pool(name="cross", bufs=2 * GRP + 1))
            # tensors that live within a window iteration
            psh = ctx.enter_context(tc.tile_pool(name="short", bufs=GRP + 1))
            # short-lived intra-phase tensors
            ptm = ctx.enter_context(tc.tile_pool(name="tmp", bufs=2))
            pl2 = ctx.enter_context(tc.tile_pool(name="l2in", bufs=2))
            pf = ctx.enter_context(tc.tile_pool(name="fin", bufs=2))
            psm = ctx.enter_context(tc.tile_pool(name="psm", bufs=3, space="PSUM"))
            ps2 = ctx.enter_context(tc.tile_pool(name="ps2", bufs=2, space="PSUM"))

            # --- persistent weights ---
            w1mu = pw.tile([128, 2 * NUM_RF], BF16, tag="w1mu")
            w1var = pw.tile([128, 2 * NUM_RF], BF16, tag="w1var")
            for k in range(2):
                nc.sync.dma_start(w1mu[:, k * NUM_RF:(k + 1) * NUM_RF], w1mu_d[k])
                nc.sync.dma_start(w1var[:, k * NUM_RF:(k + 1) * NUM_RF], w1var_d[k])
            w2mu = pw.tile([128, RFB * D_OUT], BF16, tag="w2mu")
            wa = pw.tile([128, RFB * D_OUT], BF16, tag="wa")
            wbn = pw.tile([128, RFB * D_OUT], BF16, tag="wbn")
            for b in range(RFB):
                nc.sync.dma_start(w2mu[:, b * D_OUT:(b + 1) * D_OUT], w2mu_d[b])
                nc.sync.dma_start(wa[:, b * D_OUT:(b + 1) * D_OUT], wa_d[b])
                nc.sync.dma_start(wbn[:, b * D_OUT:(b + 1) * D_OUT], wbn_d[b])

            def w1slice(w, k, b):
                return w[:, k * NUM_RF + b * 128: k * NUM_RF + (b + 1) * 128]

            HALF = FW // 2  # 1024

            def phase1(t):
                sl = slice(t * PT, (t + 1) * PT)
                xt = px.tile([128, 2, PT], BF16, tag="xt")
                x2t = px.tile([128, 2, PT], BF16, tag="x2t")
                for k in range(2):
                    nc.sync.dma_start(xt[:, k, :], xt_d[k, :, sl])
                    nc.sync.dma_start(x2t[:, k, :], x2t_d[k, :, sl])

                m_bf = pcr.tile([128, FW], BF16, tag="m_bf")
                v_bf = pcr.tile([128, FW], BF16, tag="v_bf")
                # v first: Ln (the head of the ACT chain) needs v_bf complete,
                # while m_bf is only needed later (z, after r)
                for half in range(2):
                    vps = psm.tile([128, 4 * PT], F32, tag="l1ps")
                    for bi in range(4):
                        b = half * 4 + bi
                        o = vps[:, bi * PT:(bi + 1) * PT]
                        nc.tensor.matmul(o, w1slice(w1var, 0, b), x2t[:, 0, :],
                                         start=True, stop=False)
                        nc.tensor.matmul(o, w1slice(w1var, 1, b), x2t[:, 1, :],
                                         start=False, stop=True)
                    # evict v: GPSIMD cannot read PSUM -> ACT h0 / DVE h1
                    if half == 0:
                        nc.scalar.copy(
                            v_bf[:, half * HALF:(half + 1) * HALF], vps[:])
                    else:
                        nc.vector.tensor_scalar(
                            v_bf[:, half * HALF:(half + 1) * HALF], vps[:],
                            1.0, None, ALU.mult)
                for half in range(2):
                    mps = psm.tile([128, 4 * PT], F32, tag="l1ps")
                    for bi in range(4):
                        b = half * 4 + bi
                        o = mps[:, bi * PT:(bi + 1) * PT]
                        nc.tensor.matmul(o, w1slice(w1mu, 0, b), xt[:, 0, :],
                                         start=True, stop=False)
                        nc.tensor.matmul(o, w1slice(w1mu, 1, b), xt[:, 1, :],
                                         start=False, stop=True)
                    # evict m half on DVE
                    nc.vector.tensor_scalar(
                        m_bf[:, half * HALF:(half + 1) * HALF], mps[:],
                        1.0, None, ALU.mult)

                L = ptm.tile([128, FW], BF16, tag="L")
                i_L = nc.scalar.activation(L[:], v_bf[:], AF.Ln, bias=1e-8)
                r = ptm.tile([128, FW], BF16, tag="r")
                i_r = nc.scalar.activation(r[:], L[:], AF.Exp, scale=-0.5)
                z = psh.tile([128, FW], BF16, tag="z")
                nc.vector.tensor_tensor(z[:], m_bf[:], r[:], ALU.mult)
                z2 = ptm.tile([128, FW], BF16, tag="z2")
                nc.vector.tensor_tensor(z2[:], z[:], z[:], ALU.mult)
                wp = pcr.tile([128, FW], BF16, tag="wp")
                nc.gpsimd.tensor_tensor(wp[:], L[:], z2[:], ALU.subtract)
                return dict(m_bf=m_bf, v_bf=v_bf, z=z, wp=wp,
                            i_L=i_L, i_r=i_r)

            def phase1b(st):
                # sphi2 = 2*sqrt(v)*phi(z): nl_exp-set op, independent of erf,
                # so it batches with Ln/r under one table load.  Emitted one
                # tile later than its wp so the ACT queue never head-of-line
                # blocks on the z/z2/wp round trip.
                sphi2 = psh.tile([128, FW], BF16, tag="sphi2")
                i_sphi = nc.scalar.activation(sphi2[:], st["wp"][:], AF.Exp,
                                              scale=0.5,
                                              bias=LN_2_OVER_SQRT2PI)
                st["sphi2"] = sphi2
                st["i_sphi"] = i_sphi

            def phase_erf(st):
                e = pcr.tile([128, FW], BF16, tag="e")
                i_e = nc.scalar.activation(e[:], st["z"][:], AF.Erf,
                                           scale=INV_SQRT2)
                st["e"] = e
                st["i_e"] = i_e

            def phase2a(t, st):
                m_bf, v_bf, e = st["m_bf"], st["v_bf"], st["e"]
                sphi2 = st["sphi2"]
                e1 = pl2.tile([128, FW], BF16, tag="e1")
                nc.vector.tensor_scalar(e1[:], e[:], 1.0, None, ALU.add)
                p2 = pl2.tile([128, FW], BF16, tag="p2")
                nc.vector.tensor_tensor(p2[:], m_bf[:], e1[:], ALU.mult)
                bb = pl2.tile([128, FW], BF16, tag="bb")
                nc.gpsimd.tensor_tensor(bb[:], e1[:], v_bf[:], ALU.mult)
                m2 = p2  # in-place: p2 has no other consumer
                nc.vector.tensor_tensor(m2[:], p2[:], sphi2[:], ALU.add)
                aa = pl2.tile([128, FW], BF16, tag="aa")
                nc.vector.tensor_tensor(aa[:], m_bf[:], m2[:], ALU.mult)
                st.update(m2=m2, aa=aa, bb=bb)

            def phase2(t, st):
                m_bf, m2, aa, bb = st["m_bf"], st["m2"], st["aa"], st["bb"]
                m2sq = pl2.tile([128, FW], BF16, tag="m2sq")
                if t % 2 == 0:
                    nc.scalar.activation(m2sq[:], m2[:], AF.Square)
                else:
                    nc.vector.tensor_tensor(m2sq[:], m2[:], m2[:], ALU.mult)

                mv = ps2.tile([128, 2, PT], F32, tag="mv")
                mo = mv[:, 0, :]
                vo = mv[:, 1, :]
                for b in range(RFB):
                    nc.tensor.matmul(mo, w2mu[:, b * D_OUT:(b + 1) * D_OUT],
                                     m2[:, b * PT:(b + 1) * PT],
                                     start=(b == 0), stop=(b == RFB - 1))
                for b in range(RFB):
                    nc.tensor.matmul(vo, wa[:, b * D_OUT:(b + 1) * D_OUT],
                                     aa[:, b * PT:(b + 1) * PT],
                                     start=(b == 0), stop=False)
                for b in range(RFB):
                    nc.tensor.matmul(vo, wa[:, b * D_OUT:(b + 1) * D_OUT],
                                     bb[:, b * PT:(b + 1) * PT],
                                     start=False, stop=False)
                for b in range(RFB):
                    nc.tensor.matmul(vo, wbn[:, b * D_OUT:(b + 1) * D_OUT],
                                     m2sq[:, b * PT:(b + 1) * PT],
                                     start=False, stop=(b == RFB - 1))
                st["mv"] = mv

            def finals(t, mv):
                W = PT
                mo = mv[:, 0, 0:W]
                vo = mv[:, 1, 0:W]
                nseg = W // K
                prec = pf.tile([128, W], F32, tag="prec")
                nc.vector.reciprocal_approx_fast(prec[:], vo)
                sp = pf.tile([128, nseg], F32, tag="sp")
                nc.vector.tensor_reduce(
                    sp[:], prec[:].rearrange("p (s e) -> p s e", e=K),
                    mybir.AxisListType.X, ALU.add)
                evt = pf.tile([128, nseg], F32, tag="evt")
                nc.vector.reciprocal_approx_fast(evt[:], sp[:])
                pm = pf.tile([128, W], F32, tag="pm")
                nc.vector.tensor_tensor(pm[:], prec[:], mo, ALU.mult)
                sm = pf.tile([128, nseg], F32, tag="sm")
                nc.vector.tensor_reduce(
                    sm[:], pm[:].rearrange("p (s e) -> p s e", e=K),
                    mybir.AxisListType.X, ALU.add)
                emt = pf.tile([128, nseg], F32, tag="emt")
                nc.gpsimd.tensor_tensor(emt[:], sm[:], evt[:], ALU.mult)
                ssl = slice(t * SEGT, t * SEGT + nseg)
                nc.sync.dma_start(ev_d[:, ssl], evt[:])
                nc.sync.dma_start(em_d[:, ssl], emt[:])

            prev_erfs = []
            prev = None
            for _pass in range(passes):
              for w0 in range(0, nt, GRP):
                  tiles = list(range(w0, min(w0 + GRP, nt)))
                  sts = {}
                  for t in tiles:
                      sts[t] = phase1(t)
                      for pe_i in prev_erfs:
                          add_dep_helper(sts[t]["i_L"].ins, pe_i.ins,
                                         sync=False, reason="act-table-order")
                  # sphi of the PREVIOUS window joins this window's nl_exp
                  # block; its phase2 then fills this window's erf hole.
                  if prev is not None:
                      for t in prev:
                          phase1b(prev[t])
                          for pe_i in prev_erfs:
                              add_dep_helper(prev[t]["i_sphi"].ins, pe_i.ins,
                                             sync=False,
                                             reason="act-table-order")
                  for t in tiles:
                      phase_erf(sts[t])
                      for t2_ in tiles:
                          add_dep_helper(sts[t]["i_e"].ins,
                                         sts[t2_]["i_r"].ins,
                                         sync=False, reason="act-table-order")
                      if prev is not None:
                          for t2_ in prev:
                              add_dep_helper(sts[t]["i_e"].ins,
                                             prev[t2_]["i_sphi"].ins,
                                             sync=False,
                                             reason="act-table-order")
                  if prev is not None:
                      for t in prev:
                          phase2a(t, prev[t])
                      for t in sorted(prev):
                          st2 = prev[t]
                          phase2(t, st2)
                          finals(t, st2["mv"])
                  prev_erfs = [sts[t]["i_e"] for t in tiles]
                  prev = sts
              # flush the last window
              if prev is not None:
                  for t in prev:
                      phase1b(prev[t])
                  for t in prev:
                      phase2a(t, prev[t])
                  for t in sorted(prev):
                      st2 = prev[t]
                      phase2(t, st2)
                      finals(t, st2["mv"])
                  prev = None

    orig = bacc.get_activation_tables
    bacc.get_activation_tables = _patched_act_tables
    try:
        nc.compile()
    finally:
        bacc.get_activation_tables = orig
    return nc


def _prep_host(X, X_idx, W1_mu, W1_var, W2_mu, W2_var, npc=NPC):
    idx = np.asarray(X_idx)
    order = np.argsort(idx, kind="stable")
    Xs = np.asarray(X, dtype=np.float32)[order]
    ncores = Xs.shape[0] // npc

    tob = lambda a: a.astype(ml_dtypes.bfloat16)
    w1mu = tob(np.ascontiguousarray(
        np.asarray(W1_mu, np.float32).T.reshape(2, 128, NUM_RF)))
    w1var = tob(np.ascontiguousarray(
        np.asarray(W1_var, np.float32).T.reshape(2, 128, NUM_RF)))
    W2_mu = np.asarray(W2_mu, np.float32)
    W2_var = np.asarray(W2_var, np.float32)
    w2mu_t = 0.5 * W2_mu.T                                # mo = 0.5*W2mu @ m2'
    wa_t = 0.5 * (W2_mu ** 2 + W2_var).T                  # vo += 0.5*WA @ (aa'+bb')
    wbn_t = -0.25 * (W2_mu ** 2).T                        # vo += -0.25*Wmu2 @ m2sq'
    tor = lambda a: np.ascontiguousarray(
        a.reshape(RFB, 128, D_OUT)).astype(ml_dtypes.bfloat16)
    w2mu, wa, wbn = tor(w2mu_t), tor(wa_t), tor(wbn_t)

    in_maps = []
    for c in range(ncores):
        Xc = Xs[c * npc:(c + 1) * npc]
        XT = np.ascontiguousarray(Xc.T)                   # [256, npc] f32
        xt32 = XT.reshape(2, 128, npc)
        xt = np.ascontiguousarray(xt32).astype(ml_dtypes.bfloat16)
        x2t = np.ascontiguousarray(xt32 * xt32).astype(ml_dtypes.bfloat16)
        in_maps.append(dict(xt=xt, x2t=x2t, w1mu=w1mu, w1var=w1var,
                            w2mu=w2mu, wa=wa, wbn=wbn))
    return in_maps


def _numpy_fallback(X, X_idx, W1_mu, W1_var, W2_mu, W2_var):
    X = np.asarray(X, np.float64)
    inv_sqrt2 = 1.0 / np.sqrt(2.0)
    inv_sqrt2pi = 1.0 / np.sqrt(2.0 * np.pi)
    try:
        from scipy.special import erf as _erf  # noqa: PLC0415
    except ImportError:
        import math  # noqa: PLC0415
        _erf = np.vectorize(math.erf)
    m = X @ np.asarray(W1_mu, np.float64).T
    v = (X * X) @ np.asarray(W1_var, np.float64).T
    s = np.sqrt(v + 1e-8)
    z = m / s
    Phi = 0.5 * (1.0 + _erf(z * inv_sqrt2))
    phi = np.exp(-0.5 * z * z) * inv_sqrt2pi
    m2 = m * Phi + s * phi
    ey2 = (m * m + v) * Phi + m * s * phi
    v2 = np.maximum(ey2 - m2 * m2, 1e-6)
    W2_mu = np.asarray(W2_mu, np.float64)
    W2_var = np.asarray(W2_var, np.float64)
    mo = m2 @ W2_mu.T
    vo = v2 @ (W2_mu * W2_mu).T + (m2 * m2 + v2) @ W2_var.T
    prec = 1.0 / vo
    segP = np.zeros((S, D_OUT))
    segM = np.zeros((S, D_OUT))
    np.add.at(segP, np.asarray(X_idx), prec)
    np.add.at(segM, np.asarray(X_idx), prec * mo)
    ev = 1.0 / segP
    em = segM * ev
    return em.astype(np.float32), ev.astype(np.float32)


_CACHED = {}


def kernel(X, X_idx, W1_mu, W1_var, W2_mu, W2_var):
    idx = np.asarray(X_idx)
    counts = np.bincount(idx.astype(np.int64), minlength=S)
    if len(counts) != S or counts.min() != K or counts.max() != K:
        return _numpy_fallback(X, X_idx, W1_mu, W1_var, W2_mu, W2_var)

    if "nc" not in _CACHED:
        _CACHED["nc"] = build_program()
    nc = _CACHED["nc"]

    in_maps = _prep_host(X, X_idx, W1_mu, W1_var, W2_mu, W2_var)
    res = bass_utils.run_bass_kernel_spmd(nc, in_maps,
                                          core_ids=list(range(NCORES)))
    em = np.concatenate([r["em"].T for r in res.results], axis=0)
    ev = np.concatenate([r["ev"].T for r in res.results], axis=0)
    em = np.ascontiguousarray(em, dtype=np.float32)
    ev = np.ascontiguousarray(ev, dtype=np.float32)
    return em, ev

